# revision 17
# baseline (speedup 1.0000x reference)
"""Single-layer LSTM (T=24, batch=1, I=H=2048) on 8 TRN2 NeuronCores.

Strategy (tensor-parallel over the 4H gate dimension):
- Each core k owns 1/8 of the gate rows of W_ih/W_hh: the rows for
  H-chunk [256k, 256k+256) of all four gates (strip order i, f, o, g).
- The W_hh shard (4MB bf16) stays SBUF-resident; the per-step matvec
  h @ W_hh_shard.T runs on the PE with the weights as the *moving*
  operand and 4-way column tiling (tile_position) => 4 concurrent
  streams; gates land in psum rows {0,32,64,96}.
- Input-side pre-activations xg = x @ W_ih_shard.T + b are precomputed
  once; per step they are injected into the recurrent PSUM accumulation
  group via a one-hot matmul (lhsT = e_t).
- Gates: psum -> sbuf (ACT copy), two PE 128x128 transposes flip H onto
  partitions, then the pointwise tail runs on [128, 2] tiles.
- Per-step h exchange across the 8 cores, two interchangeable backends:
    * "rdma": 7 remote_dma_broadcast sends (SBUF->SBUF). XOR-slot
      layout: slot d of every core's h buffer holds the chunk of core
      k^d; the host pre-permutes W_hh columns per core to match.
      Arrival sync via 7 per-slot semaphores (+2 per round each),
      waits attached to consuming matmuls after Tile scheduling.
    * "cc": ncfw AllGather collective through a DRAM bounce (rank-major
      slot layout). Slower floor (~5us/step) but uses separate
      hardware (TOPSP SDMA rings).
  kernel() probes the rdma path with a tiny NEFF at first call and
  falls back to "cc" if the fabric misbehaves.
"""

import numpy as np

import concourse.bacc as bacc
import concourse.mybir as mybir
import concourse.tile as tile
from concourse.bass_utils import run_bass_kernel_spmd

F32 = mybir.dt.float32
BF16 = mybir.dt.bfloat16

T = 24          # timesteps
H = 2048        # hidden size
NC = 8          # cores
HL = H // NC    # 256, per-core H chunk
GL = 4 * HL     # 1024, per-core gate rows
NCH = 16        # contraction chunks of 128
SOFF = [0, 2048, 6144, 4096]   # PyTorch gate-row base for strips (i, f, o, g)

_CACHE = {}


def _chunk_core(mode, k, d):
    """Which core's H-chunk sits in slot d of core k's h buffer."""
    return (k ^ d) if mode == "rdma" else d


def _build_nc(mode):
    nc = bacc.Bacc("TRN2", target_bir_lowering=False, debug=False,
                   num_devices=NC)
    Sig = mybir.ActivationFunctionType.Sigmoid
    Tanh = mybir.ActivationFunctionType.Tanh

    whh_d = nc.dram_tensor("whh", [NCH * 128, GL], BF16, kind="ExternalInput")
    wih_d = nc.dram_tensor("wih", [NCH * 128, GL], BF16, kind="ExternalInput")
    xT_d = nc.dram_tensor("xT", [NCH * 128, T], BF16, kind="ExternalInput")
    h0_d = nc.dram_tensor("h0", [128, 16], BF16, kind="ExternalInput")
    c0_d = nc.dram_tensor("c0", [128, 2], F32, kind="ExternalInput")
    biasrow_d = nc.dram_tensor("biasrow", [128, GL], BF16, kind="ExternalInput")
    e0_d = nc.dram_tensor("e0", [128, T], BF16, kind="ExternalInput")
    onehot_d = nc.dram_tensor("onehot", [128, 32 * T], BF16, kind="ExternalInput")
    ident_d = nc.dram_tensor("ident", [128, 128], F32, kind="ExternalInput")
    hist_d = nc.dram_tensor("hist", [128, 2 * T], F32, kind="ExternalOutput")
    cout_d = nc.dram_tensor("cout", [128, 2], F32, kind="ExternalOutput")

    if mode == "rdma":
        # Per-slot arrival semaphores: rsems[d] on core r is only ever
        # incremented by core r^d's broadcasts (+2 per round) — a sound
        # per-channel cumulative count. A single shared semaphore is NOT
        # sound: a fast peer's round-(n+1) increments can mask a slow
        # peer's missing round-n arrival.
        rsems = {d: nc.alloc_semaphore(f"rsem{d}") for d in range(1, 8)}
        lsem = nc.alloc_semaphore("lsem")
    waits = []  # (BassInstruction, sem, cumulative value)

    with tile.TileContext(nc) as tc:
        with (
            tc.tile_pool(name="persist", bufs=1) as pp,
            tc.tile_pool(name="step", bufs=2) as sp,
            tc.tile_pool(name="psg", bufs=2, space="PSUM") as pgp,
            tc.tile_pool(name="psT", bufs=2, space="PSUM") as ptp,
            tc.tile_pool(name="psxg", bufs=1, space="PSUM") as pxp,
            tc.tile_pool(name="dram", bufs=2, space="DRAM") as dp,
        ):
            whh = pp.tile([128, NCH * GL], BF16)   # col cb*GL + gl
            wih = pp.tile([128, NCH * GL], BF16)
            xT = pp.tile([128, NCH * T], BF16)     # col cb*T + t
            hbuf = pp.tile([128, 32], BF16)        # two bufs of 16 cols
            cst = pp.tile([128, 2], F32)
            xg = pp.tile([128, GL], BF16)
            biasrow = pp.tile([128, GL], BF16)
            e0 = pp.tile([128, T], BF16)
            onehot = pp.tile([128, 32 * T], BF16)
            ident = pp.tile([128, 128], F32)
            hist = pp.tile([128, 2 * T], F32)

            # ---- loads ----
            for cb in range(NCH):
                nc.sync.dma_start(whh[:, cb * GL:(cb + 1) * GL],
                                  whh_d[cb * 128:(cb + 1) * 128, :])
            for cb in range(NCH):
                nc.sync.dma_start(wih[:, cb * GL:(cb + 1) * GL],
                                  wih_d[cb * 128:(cb + 1) * 128, :])
                nc.sync.dma_start(xT[:, cb * T:(cb + 1) * T],
                                  xT_d[cb * 128:(cb + 1) * 128, :])
            nc.gpsimd.memset(hbuf[:, 16:32], 0.0)
            nc.sync.dma_start(hbuf[:, 0:16], h0_d[:, :])
            nc.sync.dma_start(cst[:, :], c0_d[:, :])
            nc.sync.dma_start(biasrow[:, :], biasrow_d[:, :])
            nc.sync.dma_start(e0[:, :], e0_d[:, :])
            nc.sync.dma_start(onehot[:, :], onehot_d[:, :])
            nc.sync.dma_start(ident[:, :], ident_d[:, :])
            nc.gpsimd.memset(xg[:, :], 0.0)

            # ---- xg precompute: xg[t, gl] = sum_j x[t,j] wih[j, gl] + b ----
            pxg = pxp.tile([128, GL], F32)
            for half in range(2):
                lo, hi = half * 512, (half + 1) * 512
                for cb in range(NCH):
                    nc.tensor.matmul(pxg[0:T, lo:hi],
                                     lhsT=xT[:, cb * T:(cb + 1) * T],
                                     rhs=wih[:, cb * GL + lo:cb * GL + hi],
                                     start=(cb == 0), stop=False)
                nc.tensor.matmul(pxg[0:T, lo:hi], lhsT=e0[:, :],
                                 rhs=biasrow[:, lo:hi],
                                 start=False, stop=True)
            nc.vector.tensor_copy(xg[0:T, :], pxg[0:T, :])  # f32 -> bf16

            # ---- recurrence ----
            for t in range(1, T + 1):
                hb = (t - 1) % 2   # read buffer
                wb = t % 2         # write buffer
                psg = pgp.tile([128, HL], F32)
                if mode == "rdma" and t >= 2:
                    # Arrival gate: waits attached directly to matmuls do
                    # NOT protect the LDWEIGHTS half of the pair (the PE
                    # pulls weight loads ahead), so carry the 7 per-slot
                    # waits on a DVE self-copy of the remote columns; the
                    # matmuls then RAW-depend on it through Tile, whose
                    # cross-engine semaphores gate the whole LDW+MM pair.
                    touch = nc.vector.tensor_copy(
                        hbuf[:, 16 * hb + 2:16 * hb + 16],
                        hbuf[:, 16 * hb + 2:16 * hb + 16])
                    for d in range(1, 8):
                        waits.append((touch, rsems[d], 2 * (t - 1)))
                for s in range(4):
                    nc.tensor.matmul(
                        psg[32 * s:32 * s + 32, :],
                        lhsT=onehot[:, 32 * (t - 1):32 * t],
                        rhs=xg[:, HL * s:HL * (s + 1)],
                        start=True, stop=False,
                        skip_group_check=True,
                        tile_position=(0, 32 * s))
                for cb in range(NCH):
                    hcol = hbuf[:, 16 * hb + cb:16 * hb + cb + 1]
                    for s in range(4):
                        nc.tensor.matmul(
                            psg[32 * s:32 * s + 1, :],
                            lhsT=hcol,
                            rhs=whh[:, cb * GL + HL * s:cb * GL + HL * (s + 1)],
                            start=False, stop=(cb == NCH - 1),
                            skip_group_check=True,
                            tile_position=(0, 32 * s))

                # strips -> sbuf, halves split across ACT and DVE so the
                # copies run concurrently and T0 starts ~100ns earlier
                sraw = sp.tile([128, HL], F32, tag="sraw")
                nc.scalar.copy(sraw[:, 0:128], psg[:, 0:128])
                nc.vector.tensor_copy(sraw[:, 128:256], psg[:, 128:256])
                # two PE transposes: H onto partitions
                psT = ptp.tile([128, HL], F32)
                for m in range(2):
                    nc.tensor.transpose(psT[:, 128 * m:128 * (m + 1)],
                                        sraw[:, 128 * m:128 * (m + 1)],
                                        ident[:, :])
                # pointwise tail. Op order tuned for engine overlap:
                # sigmoid first so fc (DVE, needs only f and c) runs while
                # tanh(g) is still on ACT; the exchange-buffer h write
                # precedes the history write so the broadcast fires as
                # early as possible.
                psT3 = psT[:].rearrange("p (m c) -> p m c", m=2)
                sact = sp.tile([128, 6], F32, tag="sact")   # col 3m+s (i,f,o)
                sact3 = sact[:].rearrange("p (m c) -> p m c", m=2)
                nc.scalar.activation(sact3[:, :, :], psT3[:, :, 0:96:32], Sig)
                fc = sp.tile([128, 2], F32, tag="fc")
                nc.vector.tensor_mul(fc[:, :], sact3[:, :, 1], cst[:, :])
                tg = sp.tile([128, 2], F32, tag="tg")
                nc.scalar.activation(tg[:, :], psT3[:, :, 96], Tanh)
                ig = sp.tile([128, 2], F32, tag="ig")
                nc.vector.tensor_mul(ig[:, :], sact3[:, :, 0], tg[:, :])
                nc.vector.tensor_add(cst[:, :], fc[:, :], ig[:, :])
                tct = sp.tile([128, 2], F32, tag="tct")
                nc.scalar.activation(tct[:, :], cst[:, :], Tanh)
                if mode == "rdma":
                    own = hbuf[:, 16 * wb:16 * wb + 2]
                    nc.vector.tensor_mul(own, sact3[:, :, 2], tct[:, :])
                nc.vector.tensor_mul(hist[:, t - 1:2 * T:T],
                                     sact3[:, :, 2], tct[:, :])
                if mode == "rdma":
                    if t < T:
                        for d in range(1, 8):
                            rdests = [None] * 8
                            # ucode lane map: slots with bit2 set deliver
                            # to tpb ^ 2
                            rdests[d] = (0, d ^ 2 if d & 4 else d)
                            nc.gpsimd.remote_dma_broadcast(
                                out_ap=hbuf[:, 16 * wb + 2 * d:
                                            16 * wb + 2 * d + 2],
                                in_ap=own,
                                remote_sem=rsems[d],
                                local_sem=lsem,
                                rdests=rdests)
                        nc.gpsimd.trigger_dma(count=None)
                else:
                    if t < T:
                        hown = sp.tile([128, 2], F32, tag="hown")
                        nc.vector.tensor_mul(hown[:, :],
                                             sact3[:, :, 2], tct[:, :])
                        agin = dp.tile([128, 2], F32, tag="agin")
                        agout = dp.tile([NC * 128, 2], F32, tag="agout")
                        nc.gpsimd.dma_start(agin[:], hown[:, :])
                        nc.gpsimd.collective_compute(
                            "AllGather", mybir.AluOpType.bypass,
                            replica_groups=[list(range(NC))],
                            ins=[agin.opt()], outs=[agout.opt()])
                        # scatter rank blocks into bf16 h columns
                        # (casting DMA: gpsimd only)
                        for d in range(NC):
                            nc.gpsimd.dma_start(
                                hbuf[:, 16 * wb + 2 * d:16 * wb + 2 * d + 2],
                                agout[128 * d:128 * (d + 1), :])

            nc.sync.dma_start(hist_d[:, :], hist[:, :])
            nc.sync.dma_start(cout_d[:, :], cst[:, :])

    # cross-core arrival waits (attached post-scheduling: Tile's
    # schedule-time simulator cannot see remote increments)
    for mm, sem, val in waits:
        mm.wait_op(sem, val, "sem-ge", check=False)
    nc.compile()
    return nc


def _build_probe():
    """Tiny NEFF exercising the same rdma XOR exchange; used for health check."""
    nc = bacc.Bacc("TRN2", target_bir_lowering=False, debug=False,
                   num_devices=NC)
    x = nc.dram_tensor("x", [128, 2], F32, kind="ExternalInput")
    out = nc.dram_tensor("out", [128, 16], F32, kind="ExternalOutput")
    rsems = {d: nc.alloc_semaphore(f"rsem{d}") for d in range(1, 8)}
    lsem = nc.alloc_semaphore("lsem")
    with tile.TileContext(nc) as tc:
        with tc.tile_pool(name="p", bufs=1) as pool:
            hbuf = pool.tile([128, 16], F32)
            nc.gpsimd.memset(hbuf[:], -1.0)
            nc.sync.dma_start(hbuf[:, 0:2], x[:, :])
            for d in range(1, 8):
                rdests = [None] * 8
                rdests[d] = (0, d ^ 2 if d & 4 else d)
                nc.gpsimd.remote_dma_broadcast(
                    out_ap=hbuf[:, 2 * d:2 * d + 2], in_ap=hbuf[:, 0:2],
                    remote_sem=rsems[d], local_sem=lsem, rdests=rdests)
            nc.gpsimd.trigger_dma(count=None)
            res = pool.tile([128, 16], F32, tag="res")
            cp = nc.vector.tensor_copy(res[:], hbuf[:])
            nc.sync.dma_start(out[:], res[:])
    for d in range(1, 8):
        cp.wait_op(rsems[d], 2, "sem-ge", check=False)
    nc.compile()
    return nc


def _rdma_healthy():
    """Require two consecutive clean runs — the fabric can be flaky."""
    try:
        nc = _build_probe()
        for _ in range(2):
            in_maps = [{"x": np.full((128, 2), float(k), np.float32)}
                       for k in range(NC)]
            res = run_bass_kernel_spmd(nc, in_maps, core_ids=list(range(NC)))
            for k in range(NC):
                got = res.results[k]["out"]
                for d in range(8):
                    if not np.all(got[:, 2 * d:2 * d + 2] == float(k ^ d)):
                        return False
        return True
    except Exception:
        return False


def _get_nc():
    if "nc" not in _CACHE:
        mode = "rdma" if _rdma_healthy() else "cc"
        _CACHE["mode"] = mode
        _CACHE["nc"] = _build_nc(mode)
    return _CACHE["nc"], _CACHE["mode"]


def _prep_core_inputs(mode, k, x, hn, cn, w_ih, w_hh, bias):
    import ml_dtypes
    b16 = ml_dtypes.bfloat16
    gg = np.concatenate(
        [SOFF[s] + HL * k + np.arange(HL) for s in range(4)])
    whh_g = w_hh[gg, :]                      # [1024, 2048]
    whh_l = np.empty((NCH * 128, GL), np.float32)
    for cb in range(NCH):
        d, a = cb // 2, cb % 2
        src = _chunk_core(mode, k, d)
        j = HL * src + 128 * a + np.arange(128)
        whh_l[cb * 128:(cb + 1) * 128, :] = whh_g[:, j].T
    wih_l = w_ih[gg, :].T                    # [2048, 1024]
    xT = x[:, 0, :].T                        # [2048, 24]
    h0 = np.empty((128, 16), np.float32)
    for d in range(8):
        src = _chunk_core(mode, k, d)
        h0[:, 2 * d:2 * d + 2] = hn[0, 0, HL * src:HL * src + HL] \
            .reshape(2, 128).T
    c0 = cn[0, 0, HL * k:HL * k + HL].reshape(2, 128).T.copy()
    biasrow = np.zeros((128, GL), np.float32)
    biasrow[0, :] = bias[gg]
    e0 = np.zeros((128, T), np.float32)
    e0[0, :] = 1.0
    onehot = np.zeros((128, 32 * T), np.float32)
    for t in range(T):
        onehot[t, 32 * t:32 * t + 32] = 1.0
    return {
        "whh": whh_l.astype(b16),
        "wih": wih_l.astype(b16),
        "xT": np.ascontiguousarray(xT).astype(b16),
        "h0": h0.astype(b16),
        "c0": np.ascontiguousarray(c0),
        "biasrow": biasrow.astype(b16),
        "e0": e0.astype(b16),
        "onehot": onehot.astype(b16),
        "ident": np.eye(128, dtype=np.float32),
    }


def kernel(input, hn, cn, w_ih, w_hh, b_ih, b_hh):
    x = np.asarray(input, np.float32)
    hn = np.asarray(hn, np.float32)
    cn = np.asarray(cn, np.float32)
    w_ih = np.asarray(w_ih, np.float32)
    w_hh = np.asarray(w_hh, np.float32)
    bias = np.asarray(b_ih, np.float32) + np.asarray(b_hh, np.float32)

    nc, mode = _get_nc()
    try:
        in_maps = [_prep_core_inputs(mode, k, x, hn, cn, w_ih, w_hh, bias)
                   for k in range(NC)]
        res = run_bass_kernel_spmd(nc, in_maps, core_ids=list(range(NC)))
    except Exception:
        if mode == "cc":
            raise
        # rdma execution failed (e.g. fabric wedged after the health
        # probe passed) — fall back to the collective backend.
        _CACHE["mode"] = mode = "cc"
        _CACHE["nc"] = nc = _build_nc("cc")
        in_maps = [_prep_core_inputs(mode, k, x, hn, cn, w_ih, w_hh, bias)
                   for k in range(NC)]
        res = run_bass_kernel_spmd(nc, in_maps, core_ids=list(range(NC)))

    out = np.empty((T, 1, H), np.float32)
    cT = np.empty((1, 1, H), np.float32)
    for k in range(NC):
        hist = res.results[k]["hist"]            # [128, 48]
        hh = hist.reshape(128, 2, T)
        out[:, 0, HL * k:HL * (k + 1)] = hh.transpose(2, 1, 0).reshape(T, HL)
        cl = res.results[k]["cout"]              # [128, 2]
        cT[0, 0, HL * k:HL * (k + 1)] = cl.T.reshape(HL)
    hT = out[T - 1:T].copy().reshape(1, 1, H)
    return out, (hT, cT)


# revision 23
# speedup vs baseline: 1.0156x; 1.0156x over previous
"""Single-layer LSTM (T=24, batch=1, I=H=2048) on 8 TRN2 NeuronCores.

Strategy (tensor-parallel over the 4H gate dimension):
- Each core k owns 1/8 of the gate rows of W_ih/W_hh: the rows for
  H-chunk [256k, 256k+256) of all four gates (strip order i, f, o, g).
- The W_hh shard (4MB bf16) stays SBUF-resident; the per-step matvec
  h @ W_hh_shard.T runs on the PE with the weights as the *moving*
  operand and 4-way column tiling (tile_position) => 4 concurrent
  streams; gates land in psum rows {0,32,64,96}.
- Input-side pre-activations xg = x @ W_ih_shard.T + b are precomputed
  once; per step they are injected into the recurrent PSUM accumulation
  group via a one-hot matmul (lhsT = e_t).
- Gates: psum -> sbuf (ACT copy), two PE 128x128 transposes flip H onto
  partitions, then the pointwise tail runs on [128, 2] tiles.
- Per-step h exchange across the 8 cores, two interchangeable backends:
    * "rdma": 7 remote_dma_broadcast sends (SBUF->SBUF). XOR-slot
      layout: slot d of every core's h buffer holds the chunk of core
      k^d; the host pre-permutes W_hh columns per core to match.
      Arrival sync via 7 per-slot semaphores (+2 per round each),
      waits attached to consuming matmuls after Tile scheduling.
    * "cc": ncfw AllGather collective through a DRAM bounce (rank-major
      slot layout). Slower floor (~5us/step) but uses separate
      hardware (TOPSP SDMA rings).
  kernel() probes the rdma path with a tiny NEFF at first call and
  falls back to "cc" if the fabric misbehaves.
"""

import numpy as np

import concourse.bacc as bacc
import concourse.mybir as mybir
import concourse.tile as tile
from concourse.bass_utils import run_bass_kernel_spmd

F32 = mybir.dt.float32
BF16 = mybir.dt.bfloat16

T = 24          # timesteps
H = 2048        # hidden size
NC = 8          # cores
HL = H // NC    # 256, per-core H chunk
GL = 4 * HL     # 1024, per-core gate rows
NCH = 16        # contraction chunks of 128
SOFF = [0, 2048, 6144, 4096]   # PyTorch gate-row base for strips (i, f, o, g)

_CACHE = {}


def _chunk_core(mode, k, d):
    """Which core's H-chunk sits in slot d of core k's h buffer."""
    return (k ^ d) if mode == "rdma" else d


def _build_nc(mode):
    nc = bacc.Bacc("TRN2", target_bir_lowering=False, debug=False,
                   num_devices=NC)
    Sig = mybir.ActivationFunctionType.Sigmoid
    Tanh = mybir.ActivationFunctionType.Tanh

    whh_d = nc.dram_tensor("whh", [NCH * 128, GL], BF16, kind="ExternalInput")
    wih_d = nc.dram_tensor("wih", [NCH * 128, GL], BF16, kind="ExternalInput")
    xT_d = nc.dram_tensor("xT", [NCH * 128, T], BF16, kind="ExternalInput")
    h0_d = nc.dram_tensor("h0", [128, 16], BF16, kind="ExternalInput")
    c0_d = nc.dram_tensor("c0", [128, 2], F32, kind="ExternalInput")
    biasrow_d = nc.dram_tensor("biasrow", [128, GL], BF16, kind="ExternalInput")
    e0_d = nc.dram_tensor("e0", [128, T], BF16, kind="ExternalInput")
    onehot_d = nc.dram_tensor("onehot", [128, 32 * T], BF16, kind="ExternalInput")
    ident_d = nc.dram_tensor("ident", [128, 128], F32, kind="ExternalInput")
    hist_d = nc.dram_tensor("hist", [128, 2 * T], F32, kind="ExternalOutput")
    cout_d = nc.dram_tensor("cout", [128, 2], F32, kind="ExternalOutput")

    if mode == "rdma":
        # Per-slot arrival semaphores: rsems[d] on core r is only ever
        # incremented by core r^d's broadcasts (+2 per round) — a sound
        # per-channel cumulative count. A single shared semaphore is NOT
        # sound: a fast peer's round-(n+1) increments can mask a slow
        # peer's missing round-n arrival.
        rsems = {d: nc.alloc_semaphore(f"rsem{d}") for d in range(1, 8)}
        lsem = nc.alloc_semaphore("lsem")
    waits = []  # (BassInstruction, sem, cumulative value)

    with tile.TileContext(nc) as tc:
        with (
            tc.tile_pool(name="persist", bufs=1) as pp,
            tc.tile_pool(name="step", bufs=2) as sp,
            tc.tile_pool(name="psg", bufs=2, space="PSUM") as pgp,
            tc.tile_pool(name="psT", bufs=2, space="PSUM") as ptp,
            tc.tile_pool(name="psxg", bufs=1, space="PSUM") as pxp,
            tc.tile_pool(name="dram", bufs=2, space="DRAM") as dp,
        ):
            whh = pp.tile([128, NCH * GL], BF16)   # col cb*GL + gl
            wih = pp.tile([128, NCH * GL], BF16)
            xT = pp.tile([128, NCH * T], BF16)     # col cb*T + t
            hbuf = pp.tile([128, 32], BF16)        # two bufs of 16 cols
            cst = pp.tile([128, 2], F32)
            xg = pp.tile([128, GL], BF16)
            biasrow = pp.tile([128, GL], BF16)
            e0 = pp.tile([128, T], BF16)
            onehot = pp.tile([128, 32 * T], BF16)
            ident = pp.tile([128, 128], F32)
            hist = pp.tile([128, 2 * T], F32)

            # ---- loads ----
            # Order matters: small tensors first (step 1 and the xg
            # matmul need them), then wih/xT (xg precompute), then whh —
            # so the xg matmul runs entirely under the whh load shadow.
            nc.gpsimd.memset(hbuf[:, 16:32], 0.0)
            nc.sync.dma_start(hbuf[:, 0:16], h0_d[:, :])
            nc.sync.dma_start(cst[:, :], c0_d[:, :])
            nc.sync.dma_start(biasrow[:, :], biasrow_d[:, :])
            nc.sync.dma_start(e0[:, :], e0_d[:, :])
            nc.sync.dma_start(onehot[:, :], onehot_d[:, :])
            nc.sync.dma_start(ident[:, :], ident_d[:, :])
            nc.gpsimd.memset(xg[:, :], 0.0)
            for cb in range(NCH):
                nc.sync.dma_start(wih[:, cb * GL:(cb + 1) * GL],
                                  wih_d[cb * 128:(cb + 1) * 128, :])
                nc.sync.dma_start(xT[:, cb * T:(cb + 1) * T],
                                  xT_d[cb * 128:(cb + 1) * 128, :])
            for cb in range(NCH):
                nc.sync.dma_start(whh[:, cb * GL:(cb + 1) * GL],
                                  whh_d[cb * 128:(cb + 1) * 128, :])

            # ---- xg precompute: xg[t, gl] = sum_j x[t,j] wih[j, gl] + b ----
            pxg = pxp.tile([128, GL], F32)
            for half in range(2):
                lo, hi = half * 512, (half + 1) * 512
                for cb in range(NCH):
                    nc.tensor.matmul(pxg[0:T, lo:hi],
                                     lhsT=xT[:, cb * T:(cb + 1) * T],
                                     rhs=wih[:, cb * GL + lo:cb * GL + hi],
                                     start=(cb == 0), stop=False)
                nc.tensor.matmul(pxg[0:T, lo:hi], lhsT=e0[:, :],
                                 rhs=biasrow[:, lo:hi],
                                 start=False, stop=True)
                # per-half f32->bf16 copy overlaps the other half's matmuls
                nc.vector.tensor_copy(xg[0:T, lo:hi], pxg[0:T, lo:hi])

            # ---- recurrence ----
            for t in range(1, T + 1):
                hb = (t - 1) % 2   # read buffer
                wb = t % 2         # write buffer
                psg = pgp.tile([128, HL], F32)
                if mode == "rdma" and t >= 2:
                    # Arrival gate: waits attached directly to matmuls do
                    # NOT protect the LDWEIGHTS half of the pair (the PE
                    # reorder window pulls weight loads past unsatisfied
                    # waits), so carry the 7 per-slot waits on a DVE
                    # self-copy of the remote columns; the matmuls then
                    # RAW-depend on it through Tile, whose cross-engine
                    # semaphores gate the whole LDW+MM pair. (A per-slot
                    # split of this touch — letting the PE ride the
                    # arrival latency gradient — showed a nondeterministic
                    # cold-run failure on HW and was reverted.)
                    touch = nc.vector.tensor_copy(
                        hbuf[:, 16 * hb + 2:16 * hb + 16],
                        hbuf[:, 16 * hb + 2:16 * hb + 16])
                    for d in range(1, 8):
                        waits.append((touch, rsems[d], 2 * (t - 1)))
                for s in range(4):
                    nc.tensor.matmul(
                        psg[32 * s:32 * s + 32, :],
                        lhsT=onehot[:, 32 * (t - 1):32 * t],
                        rhs=xg[:, HL * s:HL * (s + 1)],
                        start=True, stop=False,
                        skip_group_check=True,
                        tile_position=(0, 32 * s))
                for cb in range(NCH):
                    hcol = hbuf[:, 16 * hb + cb:16 * hb + cb + 1]
                    for s in range(4):
                        nc.tensor.matmul(
                            psg[32 * s:32 * s + 1, :],
                            lhsT=hcol,
                            rhs=whh[:, cb * GL + HL * s:cb * GL + HL * (s + 1)],
                            start=False, stop=(cb == NCH - 1),
                            skip_group_check=True,
                            tile_position=(0, 32 * s))

                # strips -> sbuf, halves split across ACT and DVE so the
                # copies run concurrently and T0 starts ~100ns earlier
                sraw = sp.tile([128, HL], F32, tag="sraw")
                nc.scalar.copy(sraw[:, 0:128], psg[:, 0:128])
                nc.vector.tensor_copy(sraw[:, 128:256], psg[:, 128:256])
                # two PE transposes: H onto partitions
                psT = ptp.tile([128, HL], F32)
                for m in range(2):
                    nc.tensor.transpose(psT[:, 128 * m:128 * (m + 1)],
                                        sraw[:, 128 * m:128 * (m + 1)],
                                        ident[:, :])
                # pointwise tail. Op order tuned for engine overlap:
                # sigmoid first so fc (DVE, needs only f and c) runs while
                # tanh(g) is still on ACT; the exchange-buffer h write
                # precedes the history write so the broadcast fires as
                # early as possible.
                psT3 = psT[:].rearrange("p (m c) -> p m c", m=2)
                sact = sp.tile([128, 6], F32, tag="sact")   # col 3m+s (i,f,o)
                sact3 = sact[:].rearrange("p (m c) -> p m c", m=2)
                nc.scalar.activation(sact3[:, :, :], psT3[:, :, 0:96:32], Sig)
                fc = sp.tile([128, 2], F32, tag="fc")
                nc.vector.tensor_mul(fc[:, :], sact3[:, :, 1], cst[:, :])
                tg = sp.tile([128, 2], F32, tag="tg")
                nc.scalar.activation(tg[:, :], psT3[:, :, 96], Tanh)
                ig = sp.tile([128, 2], F32, tag="ig")
                nc.vector.tensor_mul(ig[:, :], sact3[:, :, 0], tg[:, :])
                nc.vector.tensor_add(cst[:, :], fc[:, :], ig[:, :])
                tct = sp.tile([128, 2], F32, tag="tct")
                nc.scalar.activation(tct[:, :], cst[:, :], Tanh)
                if mode == "rdma":
                    own = hbuf[:, 16 * wb:16 * wb + 2]
                    nc.vector.tensor_mul(own, sact3[:, :, 2], tct[:, :])
                nc.vector.tensor_mul(hist[:, t - 1:2 * T:T],
                                     sact3[:, :, 2], tct[:, :])
                if mode == "rdma":
                    if t < T:
                        for d in range(1, 8):
                            rdests = [None] * 8
                            # ucode lane map: slots with bit2 set deliver
                            # to tpb ^ 2
                            rdests[d] = (0, d ^ 2 if d & 4 else d)
                            nc.gpsimd.remote_dma_broadcast(
                                out_ap=hbuf[:, 16 * wb + 2 * d:
                                            16 * wb + 2 * d + 2],
                                in_ap=own,
                                remote_sem=rsems[d],
                                local_sem=lsem,
                                rdests=rdests)
                        nc.gpsimd.trigger_dma(count=None)
                else:
                    if t < T:
                        hown = sp.tile([128, 2], F32, tag="hown")
                        nc.vector.tensor_mul(hown[:, :],
                                             sact3[:, :, 2], tct[:, :])
                        agin = dp.tile([128, 2], F32, tag="agin")
                        agout = dp.tile([NC * 128, 2], F32, tag="agout")
                        nc.gpsimd.dma_start(agin[:], hown[:, :])
                        nc.gpsimd.collective_compute(
                            "AllGather", mybir.AluOpType.bypass,
                            replica_groups=[list(range(NC))],
                            ins=[agin.opt()], outs=[agout.opt()])
                        # scatter rank blocks into bf16 h columns
                        # (casting DMA: gpsimd only)
                        for d in range(NC):
                            nc.gpsimd.dma_start(
                                hbuf[:, 16 * wb + 2 * d:16 * wb + 2 * d + 2],
                                agout[128 * d:128 * (d + 1), :])

            nc.sync.dma_start(hist_d[:, :], hist[:, :])
            nc.sync.dma_start(cout_d[:, :], cst[:, :])

    # cross-core arrival waits (attached post-scheduling: Tile's
    # schedule-time simulator cannot see remote increments)
    for mm, sem, val in waits:
        mm.wait_op(sem, val, "sem-ge", check=False)
    nc.compile()
    return nc


def _build_probe():
    """Tiny NEFF exercising the same rdma XOR exchange; used for health check."""
    nc = bacc.Bacc("TRN2", target_bir_lowering=False, debug=False,
                   num_devices=NC)
    x = nc.dram_tensor("x", [128, 2], F32, kind="ExternalInput")
    out = nc.dram_tensor("out", [128, 16], F32, kind="ExternalOutput")
    rsems = {d: nc.alloc_semaphore(f"rsem{d}") for d in range(1, 8)}
    lsem = nc.alloc_semaphore("lsem")
    with tile.TileContext(nc) as tc:
        with tc.tile_pool(name="p", bufs=1) as pool:
            hbuf = pool.tile([128, 16], F32)
            nc.gpsimd.memset(hbuf[:], -1.0)
            nc.sync.dma_start(hbuf[:, 0:2], x[:, :])
            for d in range(1, 8):
                rdests = [None] * 8
                rdests[d] = (0, d ^ 2 if d & 4 else d)
                nc.gpsimd.remote_dma_broadcast(
                    out_ap=hbuf[:, 2 * d:2 * d + 2], in_ap=hbuf[:, 0:2],
                    remote_sem=rsems[d], local_sem=lsem, rdests=rdests)
            nc.gpsimd.trigger_dma(count=None)
            res = pool.tile([128, 16], F32, tag="res")
            cp = nc.vector.tensor_copy(res[:], hbuf[:])
            nc.sync.dma_start(out[:], res[:])
    for d in range(1, 8):
        cp.wait_op(rsems[d], 2, "sem-ge", check=False)
    nc.compile()
    return nc


def _rdma_healthy():
    """Require two consecutive clean runs — the fabric can be flaky."""
    try:
        nc = _build_probe()
        for _ in range(2):
            in_maps = [{"x": np.full((128, 2), float(k), np.float32)}
                       for k in range(NC)]
            res = run_bass_kernel_spmd(nc, in_maps, core_ids=list(range(NC)))
            for k in range(NC):
                got = res.results[k]["out"]
                for d in range(8):
                    if not np.all(got[:, 2 * d:2 * d + 2] == float(k ^ d)):
                        return False
        return True
    except Exception:
        return False


def _get_nc():
    if "nc" not in _CACHE:
        mode = "rdma" if _rdma_healthy() else "cc"
        _CACHE["mode"] = mode
        _CACHE["nc"] = _build_nc(mode)
    return _CACHE["nc"], _CACHE["mode"]


def _prep_core_inputs(mode, k, x, hn, cn, w_ih, w_hh, bias):
    import ml_dtypes
    b16 = ml_dtypes.bfloat16
    gg = np.concatenate(
        [SOFF[s] + HL * k + np.arange(HL) for s in range(4)])
    whh_g = w_hh[gg, :]                      # [1024, 2048]
    whh_l = np.empty((NCH * 128, GL), np.float32)
    for cb in range(NCH):
        d, a = cb // 2, cb % 2
        src = _chunk_core(mode, k, d)
        j = HL * src + 128 * a + np.arange(128)
        whh_l[cb * 128:(cb + 1) * 128, :] = whh_g[:, j].T
    wih_l = w_ih[gg, :].T                    # [2048, 1024]
    xT = x[:, 0, :].T                        # [2048, 24]
    h0 = np.empty((128, 16), np.float32)
    for d in range(8):
        src = _chunk_core(mode, k, d)
        h0[:, 2 * d:2 * d + 2] = hn[0, 0, HL * src:HL * src + HL] \
            .reshape(2, 128).T
    c0 = cn[0, 0, HL * k:HL * k + HL].reshape(2, 128).T.copy()
    biasrow = np.zeros((128, GL), np.float32)
    biasrow[0, :] = bias[gg]
    e0 = np.zeros((128, T), np.float32)
    e0[0, :] = 1.0
    onehot = np.zeros((128, 32 * T), np.float32)
    for t in range(T):
        onehot[t, 32 * t:32 * t + 32] = 1.0
    return {
        "whh": whh_l.astype(b16),
        "wih": wih_l.astype(b16),
        "xT": np.ascontiguousarray(xT).astype(b16),
        "h0": h0.astype(b16),
        "c0": np.ascontiguousarray(c0),
        "biasrow": biasrow.astype(b16),
        "e0": e0.astype(b16),
        "onehot": onehot.astype(b16),
        "ident": np.eye(128, dtype=np.float32),
    }


def kernel(input, hn, cn, w_ih, w_hh, b_ih, b_hh):
    x = np.asarray(input, np.float32)
    hn = np.asarray(hn, np.float32)
    cn = np.asarray(cn, np.float32)
    w_ih = np.asarray(w_ih, np.float32)
    w_hh = np.asarray(w_hh, np.float32)
    bias = np.asarray(b_ih, np.float32) + np.asarray(b_hh, np.float32)

    nc, mode = _get_nc()
    try:
        in_maps = [_prep_core_inputs(mode, k, x, hn, cn, w_ih, w_hh, bias)
                   for k in range(NC)]
        res = run_bass_kernel_spmd(nc, in_maps, core_ids=list(range(NC)))
    except Exception:
        if mode == "cc":
            raise
        # rdma execution failed (e.g. fabric wedged after the health
        # probe passed) — fall back to the collective backend.
        _CACHE["mode"] = mode = "cc"
        _CACHE["nc"] = nc = _build_nc("cc")
        in_maps = [_prep_core_inputs(mode, k, x, hn, cn, w_ih, w_hh, bias)
                   for k in range(NC)]
        res = run_bass_kernel_spmd(nc, in_maps, core_ids=list(range(NC)))

    out = np.empty((T, 1, H), np.float32)
    cT = np.empty((1, 1, H), np.float32)
    for k in range(NC):
        hist = res.results[k]["hist"]            # [128, 48]
        hh = hist.reshape(128, 2, T)
        out[:, 0, HL * k:HL * (k + 1)] = hh.transpose(2, 1, 0).reshape(T, HL)
        cl = res.results[k]["cout"]              # [128, 2]
        cT[0, 0, HL * k:HL * (k + 1)] = cl.T.reshape(HL)
    hT = out[T - 1:T].copy().reshape(1, 1, H)
    return out, (hT, cT)
